# revision 25
# baseline (speedup 1.0000x reference)
"""MoE minGRU layer for Trainium2, 8 NeuronCores.

Problem: nn_MoEMinGRULayer (B=4, S=2048, D=1024, M=4 experts, top-2 router).

The axon tunnel to the device is ~15-60 MB/s, so the design minimizes
host<->device bytes per call:

- Router (0.03% of FLOPs) runs on host in exact f32; only the per-core
  combine weight for "this core's expert" is uploaded (16 KB/core).
- Sharding: cores 0-3 own batch group {0,1}, cores 4-7 own {2,3}; core
  g*4+q computes expert q for all 4096 tokens of its group. Each core
  uploads a UNIQUE quarter of x (int8 with per-(quarter,d) scales),
  pre-transposed to [D, tokens]; an on-device AllGather within each
  group of 4 reconstructs the group's full x. The minGRU scan is
  sequence-local to a core, so no comm in the recurrence.
- Each core writes its expert's router-weighted partial h into a DRAM
  bounce [4096, D] f32; a ReduceScatter(add) over the group combines the
  4 experts and leaves each core with a unique [1024, D] output quarter.
  That quarter is quantized on-device to int8 with a per-token scale
  (downloaded alongside, 1 MB + 4 KB per core).
- Expert weights (f16) and biases are device-resident across calls,
  keyed by checksum; output buffers are cached device arrays.

Per-call tunnel traffic: ~8.3 MB up + ~2.1 MB down (vs ~370 MB for the
naive full-I/O scheme).
"""

import os
import zlib
import numpy as np

X_INT8 = os.environ.get("KERNEL_X_INT8", "1") == "1"

B, S, D, M = 4, 2048, 1024, 4
T = 2 * S            # tokens per group (2 batches)
TQ = T // 4          # tokens per core quarter (= RS shard)
KC = D // 128        # contraction chunks
ET = D // 128        # expert-dim tiles
TCH = 512            # tokens per chunk
NCH = T // TCH       # chunks per core
JT = TCH // 128      # 128-token subtiles per chunk
CH_PER_SEQ = S // TCH  # chunks per sequence (scan restarts here)
TT = TQ // 128       # 128-token tiles per output quarter
GROUPS = [[0, 1, 2, 3], [4, 5, 6, 7]]

LAST_RESULT = None   # BassKernelResults of the most recent run (for test.py)
_PROG_CACHE = {}


def _build_program():
    from contextlib import ExitStack

    import concourse.bacc as bacc
    import concourse.mybir as mybir
    import concourse.tile as tile
    from concourse.masks import make_identity

    F32 = mybir.dt.float32
    F16 = mybir.dt.float16
    I8 = mybir.dt.int8
    AF = mybir.ActivationFunctionType
    OP = mybir.AluOpType

    nc = bacc.Bacc("TRN2", target_bir_lowering=False, num_devices=8)

    assert X_INT8, "packed-I/O program requires the int8 x path"
    # x rows [0, D) = int8 activations; rows [D, D+32) = 32KB of packed f32
    # aux (xsc dequant scales [128,32] then wtok router weights [128,32]),
    # so the whole volatile upload is ONE tensor (one RPC on the tunnel).
    x_d = nc.declare_dram_parameter("x", [D + 32, TQ], I8, isOutput=False)
    wg_d = nc.declare_dram_parameter("wg", [D, D], F16, isOutput=False)
    wv_d = nc.declare_dram_parameter("wv", [D, D], F16, isOutput=False)
    wd_d = nc.declare_dram_parameter("wd", [D, D], F16, isOutput=False)
    bg_d = nc.declare_dram_parameter("bg", [D], F32, isOutput=False)
    bv_d = nc.declare_dram_parameter("bv", [D], F32, isOutput=False)
    bd_d = nc.declare_dram_parameter("bd", [D], F32, isOutput=False)
    # out rows [0, TQ) = int8 output quarter; rows [TQ, TQ+4) = the per-token
    # f32 scales (4KB), so the download is also ONE tensor.
    out_d = nc.declare_dram_parameter("out", [TQ + 4, D], I8, isOutput=True)

    with ExitStack() as ctx:
        tc = ctx.enter_context(tile.TileContext(nc))
        consts = ctx.enter_context(tc.tile_pool(name="consts", bufs=1))
        wpool = ctx.enter_context(tc.tile_pool(name="w", bufs=1))
        xtp = ctx.enter_context(tc.tile_pool(name="xt", bufs=1))
        inter = ctx.enter_context(tc.tile_pool(name="inter", bufs=2))
        hpool = ctx.enter_context(tc.tile_pool(name="h", bufs=12))
        carryp = ctx.enter_context(tc.tile_pool(name="carry", bufs=2))
        outst = ctx.enter_context(tc.tile_pool(name="outst", bufs=2))
        oqp = ctx.enter_context(tc.tile_pool(name="oq", bufs=2))
        psmm = ctx.enter_context(tc.tile_pool(name="psmm", bufs=2, space="PSUM"))
        pstr = ctx.enter_context(tc.tile_pool(name="pstr", bufs=2, space="PSUM"))
        dram = ctx.enter_context(tc.tile_pool(name="dram", bufs=1, space="DRAM"))

        ident = consts.tile([128, 128], F32, tag="ident", name="ident")
        make_identity(nc, ident)

        # Bounce x into local DRAM and gather the group's 4 quarters:
        # xg rows [q*D, (q+1)*D) = quarter q as [D, TQ] (host pre-transposed).
        x_bnc = dram.tile([D, TQ], I8)
        xg = dram.tile([4 * D, TQ], I8)
        nc.gpsimd.dma_start(x_bnc[:], x_d[0:D, :])
        nc.gpsimd.collective_compute(
            "AllGather", mybir.AluOpType.bypass, replica_groups=GROUPS,
            ins=[x_bnc.opt()], outs=[xg.opt()])

        # Packed aux rows -> [128, 64] f32: cols [0,32) xsc (per-(quarter,d)
        # dequant scales), cols [32,64) wtok (token t of group = p + 128*col).
        aux_sb = consts.tile([128, 8 * KC], F32, tag="aux", name="aux")
        nc.sync.dma_start(
            out=aux_sb,
            in_=x_d.bitcast(F32)[D:D + 32, :]
            .rearrange("r (a f) -> (r a) f", a=4))
        WTOFF = 4 * KC          # wtok column offset within aux_sb

        # Biases: [e on partitions within tile, et tile index on free]
        b_sb = {}
        for nm, dram_t in (("bg", bg_d), ("bv", bv_d), ("bd", bd_d)):
            t = consts.tile([128, ET], F32, tag=nm + "s", name=nm + "s")
            nc.sync.dma_start(out=t, in_=dram_t[:].rearrange("(et p) -> p et", p=128))
            b_sb[nm] = t

        # Expert weights: [d_in on partitions (kc chunks), d_out on free].
        w_sb = {}
        for nm, dram_t in (("wg", wg_d), ("wv", wv_d), ("wd", wd_d)):
            t = wpool.tile([128, KC, D], F16, tag=nm, name=nm)
            for kc in range(KC):
                nc.sync.dma_start(out=t[:, kc, :], in_=dram_t[kc * 128:(kc + 1) * 128, :])
            w_sb[nm] = t

        def load_xt(ch):
            """x chunk from the gathered group buffer, [d on partitions, tok
            free]; dequantized to f16 with the per-(quarter,d) scale."""
            q, off = divmod(ch * TCH, TQ)
            src = (xg[q * D:(q + 1) * D, off:off + TCH]
                   .rearrange("(kc p) t -> p kc t", p=128))
            x8 = xtp.tile([128, KC, TCH], I8, tag="x8", name="x8", bufs=2)
            nc.sync.dma_start(out=x8, in_=src)
            xT = xtp.tile([128, KC, TCH], F16, tag="xT", name="xT", bufs=2)
            for kc in range(KC):
                nc.scalar.activation(xT[:, kc, :], x8[:, kc, :], AF.Copy,
                                     bias=0.0, scale=aux_sb[:, q * KC + kc: q * KC + kc + 1])
            return xT

        # Partial (this expert's weighted h) for the whole group, f32;
        # ReduceScatter(add) then hands each core its own token quarter.
        part = dram.tile([T, D], F32)
        rs_out = dram.tile([TQ, D], F32)

        osb_cur = []

        def out_stage(ch, et, h):
            """Transpose h back to [token, e], scale by the router weight into
            the per-chunk assembly tiles; store contiguously after et=7."""
            t0 = ch * TCH
            es = slice(et * 128, (et + 1) * 128)
            if et == 0:
                osb_cur.clear()
                for j in range(JT):
                    osb_cur.append(outst.tile([128, D], F32, tag=f"ob{j}", name=f"ob{j}"))
            pto = pstr.tile([128, TCH], F32, tag="tr", name="tr")
            for j in range(JT):
                nc.tensor.transpose(pto[:, j * 128:(j + 1) * 128],
                                    h[:, j * 128:(j + 1) * 128], ident)
            for j in range(JT):
                wcol = aux_sb[:, WTOFF + ch * JT + j: WTOFF + ch * JT + j + 1]
                if et % 2 == 0:
                    nc.vector.tensor_scalar_mul(osb_cur[j][:, es],
                                                pto[:, j * 128:(j + 1) * 128], wcol)
                else:
                    nc.scalar.activation(osb_cur[j][:, es], pto[:, j * 128:(j + 1) * 128],
                                         AF.Copy, bias=0.0, scale=wcol)
            if et == ET - 1:
                for j in range(JT):
                    nc.sync.dma_start(
                        out=part[t0 + j * 128:t0 + (j + 1) * 128, :],
                        in_=osb_cur[j])

        xt_next = load_xt(0)
        hcarry = [None] * ET
        h_prev = None
        for ch in range(NCH):
            seq_start = (ch % CH_PER_SEQ == 0)
            xT16 = xt_next
            if ch + 1 < NCH:
                xt_next = load_xt(ch + 1)

            # Expert projections + minGRU scan; the PREVIOUS chunk's output
            # stage is interleaved here so its h-transposes hide inside the
            # matmul spans.
            h_tiles = []
            for et in range(ET):
                pg = psmm.tile([128, TCH], F32, tag="pg", name="pg")
                pv = psmm.tile([128, TCH], F32, tag="pv", name="pv")
                pd = psmm.tile([128, TCH], F32, tag="pd", name="pd")
                es = slice(et * 128, (et + 1) * 128)
                for ps, wn in ((pg, "wg"), (pv, "wv"), (pd, "wd")):
                    for kc in range(KC):
                        nc.tensor.matmul(ps, w_sb[wn][:, kc, es], xT16[:, kc, :],
                                         start=(kc == 0), stop=(kc == KC - 1))
                gs = inter.tile([128, TCH], F32, tag="gs", name="gs")
                vt = inter.tile([128, TCH], F32, tag="vt", name="vt")
                aa = inter.tile([128, TCH], F32, tag="aa", name="aa")
                nc.scalar.activation(gs, pg, AF.Sigmoid, bias=b_sb["bg"][:, et:et + 1])
                nc.scalar.activation(vt, pv, AF.Tanh, bias=b_sb["bv"][:, et:et + 1])
                nc.scalar.activation(aa, pd, AF.Sigmoid, bias=b_sb["bd"][:, et:et + 1])
                nc.vector.tensor_scalar(aa, aa, 0.998, 0.001, OP.mult, OP.add)
                nc.vector.tensor_tensor(gs, gs, vt, OP.mult)   # x_scan, in place
                h = hpool.tile([128, TCH], F32, tag="h", name="h")
                init = 0.0 if seq_start else hcarry[et][:, 0:1]
                nc.vector.tensor_tensor_scan(h, aa, gs, init, OP.mult, OP.add)
                nhc = carryp.tile([128, 1], F32, tag=f"c{et}", name=f"c{et}")
                nc.vector.tensor_copy(nhc, h[:, TCH - 1:TCH])
                hcarry[et] = nhc
                h_tiles.append(h)
                if h_prev is not None:
                    out_stage(ch - 1, et, h_prev[et])
            h_prev = h_tiles

        # Flush the last chunk's output stage.
        for et in range(ET):
            out_stage(NCH - 1, et, h_prev[et])

        # Combine the 4 experts of the group; each core keeps its quarter.
        nc.gpsimd.collective_compute(
            "ReduceScatter", mybir.AluOpType.add, replica_groups=GROUPS,
            ins=[part.opt()], outs=[rs_out.opt()])

        # Quantize the combined quarter to int8 with per-token scales; the
        # scales ride in the out tensor's 4 tail rows (f32 bytes).
        osc_sb = oqp.tile([128, TT], F32, tag="oscs", name="oscs", bufs=1)
        for ti in range(TT):
            rss = oqp.tile([128, D], F32, tag="rss", name="rss")
            nc.sync.dma_start(out=rss, in_=rs_out[ti * 128:(ti + 1) * 128, :])
            r = oqp.tile([128, 1], F32, tag="r", name="r")
            nc.vector.tensor_reduce(r, rss, mybir.AxisListType.X, OP.max,
                                    apply_absolute_value=True)
            nc.vector.tensor_scalar(r, r, 1e-30, None, OP.max)
            nc.vector.tensor_copy(osc_sb[:, ti:ti + 1], r)
            rinv = oqp.tile([128, 1], F32, tag="rinv", name="rinv")
            nc.vector.reciprocal(rinv, r)
            nc.vector.tensor_scalar(rinv, rinv, 127.0, None, OP.mult)
            q8 = oqp.tile([128, D], I8, tag="q8", name="q8")
            nc.scalar.activation(q8, rss, AF.Copy, bias=0.0, scale=rinv[:, 0:1])
            nc.sync.dma_start(out=out_d[ti * 128:(ti + 1) * 128, :], in_=q8)
        nc.sync.dma_start(
            out=out_d.bitcast(F32)[TQ:TQ + 4, :]
            .rearrange("r (a p) -> p (r a)", p=128),
            in_=osc_sb)

    nc.compile()
    return nc


def _get_program():
    if "nc" not in _PROG_CACHE:
        _PROG_CACHE["nc"] = _build_program()
    return _PROG_CACHE["nc"]


def _checksum(*arrays):
    h = 0
    for a in arrays:
        a = np.ascontiguousarray(a)
        stride = max(1, a.size // (1 << 16))
        h = zlib.adler32(np.ascontiguousarray(a.reshape(-1)[::stride]).view(np.uint8), h)
        s = int(a.view(np.uint32).sum(dtype=np.uint64))
        h = zlib.adler32(repr((a.shape, a.dtype.str, s)).encode(), h)
    return h


def _router_host(xf, gate_W):
    """Exact f32 top-2 softmax combine weights, [B*S, M] (0 for unselected)."""
    logits = xf.reshape(B * S, D) @ np.asarray(gate_W, np.float32)
    order = np.argsort(-logits, axis=-1, kind="stable")[:, :2]
    tv = np.take_along_axis(logits, order, axis=-1)
    e = np.exp(tv - tv.max(-1, keepdims=True))
    wk = (e / e.sum(-1, keepdims=True)).astype(np.float32)
    comb = np.zeros((B * S, M), np.float32)
    np.put_along_axis(comb, order, wk, axis=-1)
    return comb


def kernel(x, Wg, bg, Wv, bv, Wd, bd, gate_W):
    import jax
    from jax.sharding import PartitionSpec, NamedSharding

    f = np.float32
    x = np.asarray(x, f)
    nc = _get_program()
    if "runner" not in _PROG_CACHE:
        _PROG_CACHE["runner"] = _make_runner(nc)
    fn, in_names, out_names, out_avals, mesh = _PROG_CACHE["runner"]
    sh = NamedSharding(mesh, PartitionSpec("core"))

    # --- host quantize + router (tiny) + single packed upload tensor ---
    xr = x.reshape(8, TQ, D)     # [half-seq, tok, d]; half-seq index == core
    if "xbuf" not in _PROG_CACHE:
        _PROG_CACHE["xbuf"] = np.empty((TQ, D), f)
        _PROG_CACHE["xfull"] = np.empty((8, D + 32, TQ), np.int8)
    buf, xfull = _PROG_CACHE["xbuf"], _PROG_CACHE["xfull"]
    # per-core blocking keeps the rint/cast passes cache-resident
    sc = np.empty((8, D), f)
    for c in range(8):
        xc = xr[c]
        np.maximum(np.maximum(xc.max(axis=0), -xc.min(axis=0)), 1e-30,
                   out=sc[c])
        np.multiply(xc, (127.0 / sc[c])[None, :], out=buf)
        np.rint(buf, out=buf)    # |q| <= 127 by construction; no clip needed
        # values are exact integers, so the casting truncation == round
        xfull[c, :D, :] = buf.T

    comb = _router_host(x, gate_W)                     # [B*S, M]
    scd = sc * (1.0 / 127.0)
    aux = np.empty((8, 128, 8 * KC), f)
    for g in range(2):
        blk = scd[4 * g:4 * g + 4].reshape(4, KC, 128).transpose(2, 0, 1)
        aux[4 * g:4 * g + 4, :, :4 * KC] = blk.reshape(128, 4 * KC)
    for c in range(8):
        g, q = divmod(c, 4)
        aux[c, :, 4 * KC:] = comb[g * T:(g + 1) * T, q].reshape(NCH * JT, 128).T
    xfull[:, D:, :] = np.ascontiguousarray(aux).view(np.int8).reshape(8, 32, TQ)
    # issue the (async) upload now so it streams while we checksum weights
    dput = {"x": jax.device_put(xfull.reshape(8 * (D + 32), TQ), sh)}

    # --- device-resident weights (re-upload only if they change) ---
    # fast path: same array objects + sampled bytes -> skip the full 48MB
    # checksum pass (it contends with the x upload for the single host CPU)
    wraw = (Wg, Wv, Wd, bg, bv, bd)
    fp = tuple((id(a), a.ctypes.data if isinstance(a, np.ndarray) else 0,
                getattr(a, "shape", None),
                zlib.adler32(np.ascontiguousarray(
                    np.asarray(a, f).reshape(-1)[:: max(1, a.size // 8192)])
                    .view(np.uint8)))
               for a in wraw)
    if _PROG_CACHE.get("wfp") == fp and "wdev" in _PROG_CACHE:
        wsum = _PROG_CACHE["wsum"]
    else:
        wsum = _checksum(np.asarray(Wg, f), np.asarray(Wv, f), np.asarray(Wd, f),
                         np.asarray(bg, f), np.asarray(bv, f), np.asarray(bd, f))
        _PROG_CACHE["wfp"] = fp
    if _PROG_CACHE.get("wsum") != wsum:
        wmap = {}
        for nm, W in (("wg", Wg), ("wv", Wv), ("wd", Wd)):
            Wf16 = np.asarray(W, f).astype(np.float16)      # [M, D, D]
            wmap[nm] = np.ascontiguousarray(
                np.concatenate([Wf16[c % 4] for c in range(8)], axis=0))
        for nm, b in (("bg", bg), ("bv", bv), ("bd", bd)):
            bf = np.asarray(b, f)
            wmap[nm] = np.ascontiguousarray(
                np.concatenate([bf[c % 4] for c in range(8)], axis=0))
        _PROG_CACHE["wdev"] = {nm: jax.device_put(v, sh) for nm, v in wmap.items()}
        _PROG_CACHE["wsum"] = wsum
    if "obuf" not in _PROG_CACHE:
        _PROG_CACHE["obuf"] = [
            jax.device_put(np.zeros((8 * a.shape[0], *a.shape[1:]), a.dtype), sh)
            for a in out_avals]
    wdev = _PROG_CACHE["wdev"]

    args = [dput[nm] if nm in dput else wdev[nm] for nm in in_names]
    args += _PROG_CACHE["obuf"]

    outs = fn(*args)
    try:
        # queue the D2H server-side so it starts the moment exec completes,
        # instead of waiting for the client's np.asarray round trip
        outs[0].copy_to_host_async()
    except Exception:
        pass
    res = np.asarray(outs[0]).reshape(8, TQ + 4, D)    # int8 + f32-byte tail
    oq = res[:, :TQ, :].reshape(B * S, D)
    rsc = np.ascontiguousarray(res[:, TQ:, :]).view(f).reshape(B * S)
    out = np.multiply(oq, (rsc * (1.0 / 127.0))[:, None], dtype=f)
    return out.reshape(B, S, D)


def _make_runner(nc, n_cores=8):
    """Cached jitted shard_map executor (mirrors run_bass_kernel_spmd's axon
    path, but reusable across calls: no re-trace / re-jit / re-compile)."""
    import jax
    from jax.sharding import Mesh, PartitionSpec
    from jax.experimental.shard_map import shard_map
    import concourse.mybir as mybir
    from concourse import bass2jax

    bass2jax.install_neuronx_cc_hook()
    pname = nc.partition_id_tensor.name if nc.partition_id_tensor else None
    in_names, out_names, out_avals = [], [], []
    for alloc in nc.m.functions[0].allocations:
        if not isinstance(alloc, mybir.MemoryLocationSet):
            continue
        name = alloc.memorylocations[0].name
        if alloc.kind == "ExternalInput":
            if name != pname:
                in_names.append(name)
        elif alloc.kind == "ExternalOutput":
            out_names.append(name)
            out_avals.append(jax.core.ShapedArray(
                tuple(alloc.tensor_shape), mybir.dt.np(alloc.dtype)))
    n_params = len(in_names)
    all_in_names = in_names + out_names + ([pname] if pname else [])

    def _body(*args):
        operands = list(args)
        if pname is not None:
            operands.append(bass2jax.partition_id_tensor())
        return tuple(bass2jax._bass_exec_p.bind(
            *operands,
            out_avals=tuple(out_avals),
            in_names=tuple(all_in_names),
            out_names=tuple(out_names),
            lowering_input_output_aliases=(),
            sim_require_finite=True,
            sim_require_nnan=True,
            nc=nc,
        ))

    devices = jax.devices()[:n_cores]
    mesh = Mesh(np.asarray(devices), ("core",))
    nspecs = n_params + len(out_names)
    fn = jax.jit(shard_map(_body,
                           mesh=mesh,
                           in_specs=(PartitionSpec("core"),) * nspecs,
                           out_specs=(PartitionSpec("core"),) * len(out_names),
                           check_rep=False))
    return fn, in_names, out_names, out_avals, mesh
